# revision 1
# baseline (speedup 1.0000x reference)
"""MetaSR super-resolution Trainium2 kernel.

Structure exploited: out_h=out_w=256 with H=W=64 LR grid means the scale
factor is exactly 4, so the nearest-neighbor gather index is iy=oy//4,
ix=ox//4 and the per-query MLP input collapses to 16 distinct subpixel
phases [dy/4, dx/4, 0.25].  The whole model becomes:

  1. h    = relu(mlp_in @ w1 + b1)              [16, 256]
  2. predw = h @ w2 + b2                        [16, 576, 3]
  3. rgb[o, 4*iy+dy, 4*ix+dx] =
       sum_{c,ki,kj} feat[c, iy+ki-1, ix+kj-1] * predw[(dy,dx), c*9+ki*3+kj, o]
     i.e. a 3x3 conv with 64 in / 48 out channels + pixel shuffle.

Sharding: data-parallel over LR rows (8 rows per core, 10-row halo band),
weights replicated; steps 1+2 are recomputed on every core (tiny).

The conv contraction (K = 9 taps x 64 ch = 576) is chunked K=128 by pairing
taps.  Each core holds the zero-padded band twice in a 128-partition tile at
free-dim offsets that differ by the two taps' shift delta, so one K=128
matmul consumes two taps without materializing the unfolded tensor:
  band free index = r*66 + x  (66-wide zero-padded rows), tap (ki,kj) shift
  = ki*66+kj; taps are paired with shift deltas 1 or 64.

Inputs are packed host-side into a few large per-core DRAM blobs, ordered by
when the kernel needs them (small weights -> first w2 chunk -> band -> rest)
so compute starts as soon as the first blob lands.  A run of dummy matmuls
(zero scratch data, overwritten by the first real conv accumulation) warms
the PE HAM clock gate during the DMA phase.

float32r mode (METASR_F32R=1): the conv matmuls run in fp32r (full-rate fp32
on the PE); band data is pre-rounded host-side and W is written as fp32r.
"""

import os

import numpy as np

try:
    import concourse.bass as bass
except ImportError:  # fall back to the repo checkout
    import sys
    sys.path.insert(0, "/opt/trn_rl_repo")
    import concourse.bass as bass
import concourse.mybir as mybir
import concourse.tile as tile
from concourse import bacc
from concourse.bass_utils import run_bass_kernel_spmd

F32 = mybir.dt.float32
F32R = mybir.dt.float32r
BF16 = mybir.dt.bfloat16
N_CORES = 8
ROWS_PER_CORE = 8          # LR rows per core
BAND_ROWS = ROWS_PER_CORE + 2
NPOS = ROWS_PER_CORE * 64  # 512 LR positions per core

# Tap order for K-chunking.  Taps t = ki*3+kj have band shift ki*66+kj:
#   t:      0   1   2   3    4    5    6    7    8
#   shift:  0   1   2   66   67   68   132  133  134
# chunk0: [t0; t1] band1 off 1 | chunk1: [t3; t2] band2 off 66
# chunk2: [t4; t5] band1 off 68 | chunk3: [t6; t7] band1 off 133
# chunk4: [t8] band2 off 134 (K=64)
TAP_ORDER = [0, 1, 3, 2, 4, 5, 6, 7, 8]
CHUNK_SPECS = [  # (band_tile_idx, rhs_offset, K)
    (0, 1, 128),
    (1, 66, 128),
    (0, 68, 128),
    (0, 133, 128),
    (1, 134, 64),
]

# blob_sm0 layout: small constants + w2 m=0 block
OFF_W1 = 0          # [3, 256]   (partitions 0-2)
OFF_MLP = 256       # [3, 16]
OFF_B1B2 = 272      # [128, 17]: cols 0-1 = b1 chunks, 2-16 = b2 (o*5+m)
OFF_M0 = 289        # w2 m=0 block: 6 sub-blocks (o*2+hc) x [128, 128]
COLS_SM0 = 289 + 768
# blob_band: band1 [128, 661] + band2 [128, 724]
OFF_BAND1 = 0
OFF_BAND2 = 661
COLS_BAND = 1385
# blob_b12: w2 m=1,2 blocks; blob_b34: m=3,4
COLS_B12 = 768 * 2
COLS_B34 = 768 + 384

N_WARMUP_MM = 5

USE_F32R = os.environ.get("METASR_F32R", "1") == "1"

_CACHE = {}


def _build_program(use_f32r):
    """Build + compile the single-core Bass program (same for all cores)."""
    nc = bacc.Bacc("TRN2", target_bir_lowering=False, debug=False)

    band_dt = F32R if use_f32r else F32
    w2_dt = F32R if use_f32r else F32
    blob_sm0_d = nc.dram_tensor(
        "blob_sm0", [128, COLS_SM0], w2_dt, kind="ExternalInput"
    )
    blob_band_d = nc.dram_tensor(
        "blob_band", [128, COLS_BAND], band_dt, kind="ExternalInput"
    )
    blob_b12_d = nc.dram_tensor(
        "blob_b12", [128, COLS_B12], w2_dt, kind="ExternalInput"
    )
    blob_b34_d = nc.dram_tensor(
        "blob_b34", [128, COLS_B34], w2_dt, kind="ExternalInput"
    )
    out48 = nc.dram_tensor("out48", [48, NPOS], F32, kind="ExternalOutput")

    with tile.TileContext(nc) as tc:
        with (
            tc.tile_pool(name="blobs", bufs=1) as blobs,
            tc.tile_pool(name="work", bufs=1) as work,
            tc.tile_pool(name="wpool", bufs=5) as wpool,
            tc.tile_pool(name="opool", bufs=1) as opool,
            tc.tile_pool(name="ps_small", bufs=2, space="PSUM") as ps_small,
            tc.tile_pool(name="ps_w", bufs=5, space="PSUM") as ps_w,
            tc.tile_pool(name="ps_rgb", bufs=1, space="PSUM") as ps_rgb,
        ):
            # 4 DMAs, 2 per HWDGE ring (ACT: sm0, b34 | SP: b12, band)
            blob_sm0 = blobs.tile([128, COLS_SM0], w2_dt, tag="blob_sm0")
            nc.scalar.dma_start(blob_sm0[:, :], blob_sm0_d[:, :])
            blob_b12 = blobs.tile([128, COLS_B12], w2_dt, tag="blob_b12")
            nc.sync.dma_start(blob_b12[:, :], blob_b12_d[:, :])
            blob_b34 = blobs.tile([128, COLS_B34], w2_dt, tag="blob_b34")
            nc.scalar.dma_start(blob_b34[:, :], blob_b34_d[:, :])
            blob_band = blobs.tile([128, COLS_BAND], band_dt, tag="blob_band")
            nc.sync.dma_start(blob_band[:, :], blob_band_d[:, :])

            sm0_f32 = blob_sm0.bitcast(F32) if use_f32r else blob_sm0
            w1_sb = sm0_f32[0:3, OFF_W1:OFF_W1 + 256]
            mlp_sb = sm0_f32[0:3, OFF_MLP:OFF_MLP + 16]
            b1b2 = sm0_f32[:, OFF_B1B2:OFF_B1B2 + 17]
            band_tiles = [
                blob_band[:, OFF_BAND1:OFF_BAND1 + 661],
                blob_band[:, OFF_BAND2:OFF_BAND2 + 724],
            ]

            def w2_slice(m, o, hc, msize):
                if m == 0:
                    base = OFF_M0 + (o * 2 + hc) * 128
                    return blob_sm0[:, base:base + msize]
                if m <= 2:
                    base = (m - 1) * 768 + (o * 2 + hc) * msize
                    return blob_b12[:, base:base + msize]
                base = (m - 3) * 768 + (o * 2 + hc) * msize
                return blob_b34[:, base:base + msize]

            # ---- PE warm-up: dummy zero matmuls into rgb_ps while DMAs run.
            # conv chunk 0 below uses start=True, which resets the PSUM
            # accumulation, so these contribute nothing to the result.
            rgb_ps = ps_rgb.tile([48, NPOS], F32, tag="rgb")
            warm = work.tile([128, 512], F32, tag="warm")
            nc.vector.memset(warm[:, :], 0.0)
            warm_bf = warm.bitcast(BF16)
            for _ in range(N_WARMUP_MM):
                nc.tensor.matmul(
                    rgb_ps[:, :], warm_bf[:, 0:48], warm_bf[:, 0:NPOS],
                    start=True, stop=True,
                )

            # ---- MLP layer 1: h_actT [256, 16] in two 128-chunks ----
            h_dt = F32R if use_f32r else F32
            h_sb = work.tile([128, 32], h_dt, tag="hact")
            for hc in range(2):
                ph = ps_small.tile([128, 16], F32, tag="ph")
                nc.tensor.matmul(
                    ph[:, :], w1_sb[:, hc * 128:(hc + 1) * 128], mlp_sb[:, :],
                    start=True, stop=True,
                )
                # relu(x + b1) = max(x + b1, 0) in one DVE op
                nc.vector.tensor_scalar(
                    h_sb[:, hc * 16:(hc + 1) * 16], ph[:, :],
                    b1b2[:, hc:hc + 1], 0.0,
                    mybir.AluOpType.add, mybir.AluOpType.max,
                )

            # ---- per K-chunk: W assembly (MLP layer 2) + conv matmul ----
            w_dt = F32R if use_f32r else F32
            for m, (bidx, roff, K) in enumerate(CHUNK_SPECS):
                msize = K
                w_sb = wpool.tile([128, 48], w_dt, tag="W")
                for o in range(3):
                    pw = ps_w.tile([128, 16], F32, tag="pw")
                    for hc in range(2):
                        nc.tensor.matmul(
                            pw[:msize, :],
                            w2_slice(m, o, hc, msize),
                            h_sb[:, hc * 16:(hc + 1) * 16],
                            start=(hc == 0), stop=(hc == 1),
                        )
                    nc.vector.tensor_scalar_add(
                        w_sb[:msize, o * 16:(o + 1) * 16], pw[:msize, :],
                        b1b2[:msize, 2 + o * 5 + m:3 + o * 5 + m],
                    )
                bt = band_tiles[bidx]
                rhs = bt[0:K, roff:roff + 8 * 66].rearrange(
                    "p (r c) -> p r c", c=66
                )[:, :, 0:64]
                nc.tensor.matmul(
                    rgb_ps[:, :], w_sb[:msize, :], rhs,
                    start=(m == 0), stop=(m == len(CHUNK_SPECS) - 1),
                )

            # ---- write out ----
            out_sb = opool.tile([48, NPOS], F32, tag="out")
            nc.vector.tensor_copy(out_sb[:, :], rgb_ps[:, :])
            nc.sync.dma_start(out48[:, :], out_sb[:, :])

    nc.compile()
    return nc


def _round_f32r(x):
    """Round fp32 to the fp32r-representable set (bf16 hi + bf16 lo pair)."""
    import ml_dtypes
    hi = x.astype(ml_dtypes.bfloat16).astype(np.float32)
    lo = (x - hi).astype(ml_dtypes.bfloat16).astype(np.float32)
    return hi + lo


def _host_prep(feat, w1, b1, w2, b2, use_f32r):
    """Pack shared blobs + per-core band blobs."""
    feat = np.ascontiguousarray(np.asarray(feat, dtype=np.float32))[0]  # [64,64,64]
    w1 = np.asarray(w1, dtype=np.float32)
    b1 = np.asarray(b1, dtype=np.float32)
    w2 = np.asarray(w2, dtype=np.float32)
    b2 = np.asarray(b2, dtype=np.float32)

    dydx = np.arange(16)
    mlpin = np.stack(
        [dydx // 4 / 4.0, dydx % 4 / 4.0, np.full(16, 0.25)], axis=0
    ).astype(np.float32)  # [3, 16]

    # tap-major permutations of w2/b2
    w2r = w2.reshape(256, 64, 9, 3)  # [h, c, t, o]
    w2p = np.empty((3, 256, 576), dtype=np.float32)
    b2r = b2.reshape(64, 9, 3)       # [c, t, o]
    b2p = np.empty((3, 576), dtype=np.float32)
    for blk, t in enumerate(TAP_ORDER):
        w2p[:, :, blk * 64:(blk + 1) * 64] = w2r[:, :, t, :].transpose(2, 0, 1)
        b2p[:, blk * 64:(blk + 1) * 64] = b2r[:, t, :].T

    if use_f32r:
        w2p = _round_f32r(w2p)

    blob_sm0 = np.zeros((128, COLS_SM0), dtype=np.float32)
    blob_sm0[0:3, OFF_W1:OFF_W1 + 256] = w1
    blob_sm0[0:3, OFF_MLP:OFF_MLP + 16] = mlpin
    blob_sm0[:, OFF_B1B2 + 0] = b1[0:128]
    blob_sm0[:, OFF_B1B2 + 1] = b1[128:256]
    for o in range(3):
        for m in range(5):
            msize = 128 if m < 4 else 64
            blob_sm0[:msize, OFF_B1B2 + 2 + o * 5 + m] = \
                b2p[o, 128 * m:128 * m + msize]
    for o in range(3):
        for hc in range(2):
            base = OFF_M0 + (o * 2 + hc) * 128
            blob_sm0[:, base:base + 128] = w2p[o, hc * 128:(hc + 1) * 128, 0:128]

    blob_b12 = np.empty((128, COLS_B12), dtype=np.float32)
    blob_b34 = np.empty((128, COLS_B34), dtype=np.float32)
    for m in range(1, 5):
        msize = 128 if m < 4 else 64
        dst = blob_b12 if m <= 2 else blob_b34
        moff = (m - 1) * 768 if m <= 2 else (m - 3) * 768
        for o in range(3):
            for hc in range(2):
                base = moff + (o * 2 + hc) * msize
                dst[:, base:base + msize] = \
                    w2p[o, hc * 128:(hc + 1) * 128, 128 * m:128 * m + msize]

    featp = np.zeros((64, 66, 66), dtype=np.float32)
    featp[:, 1:65, 1:65] = feat
    if use_f32r:
        featp = _round_f32r(featp)

    blobs_band = []
    for core in range(N_CORES):
        r0 = core * ROWS_PER_CORE
        band = featp[:, r0:r0 + BAND_ROWS, :].reshape(64, BAND_ROWS * 66)
        bb = np.zeros((128, COLS_BAND), dtype=np.float32)
        bb[0:64, OFF_BAND1 + 1:OFF_BAND1 + 661] = band
        bb[64:128, OFF_BAND1 + 0:OFF_BAND1 + 660] = band
        bb[0:64, OFF_BAND2 + 0:OFF_BAND2 + 660] = band
        bb[64:128, OFF_BAND2 + 64:OFF_BAND2 + 724] = band
        blobs_band.append(bb)
    return blob_sm0, blob_b12, blob_b34, blobs_band


def _assemble(per_core_out48):
    """[8 x [48, 512]] -> [1, 3, 256, 256]."""
    full = np.stack(per_core_out48)                      # [core, 48, 512]
    full = full.reshape(8, 3, 4, 4, 8, 64)               # [core, o, dy, dx, r, x]
    rgb = full.transpose(1, 0, 4, 2, 5, 3).reshape(3, 256, 256)
    return np.ascontiguousarray(rgb)[None]


def get_program():
    key = ("nc", USE_F32R)
    if key not in _CACHE:
        _CACHE[key] = _build_program(USE_F32R)
    return _CACHE[key]


def run(feat, w1, b1, w2, b2, out_h, out_w, trace=False, **spmd_kwargs):
    assert int(out_h) == 256 and int(out_w) == 256
    nc = get_program()
    blob_sm0, blob_b12, blob_b34, blobs_band = _host_prep(
        feat, w1, b1, w2, b2, USE_F32R
    )
    in_maps = [
        {"blob_sm0": blob_sm0, "blob_b12": blob_b12, "blob_b34": blob_b34,
         "blob_band": blobs_band[core]}
        for core in range(N_CORES)
    ]
    res = run_bass_kernel_spmd(
        nc, in_maps, core_ids=list(range(N_CORES)), trace=trace, **spmd_kwargs
    )
    out = _assemble([res.results[core]["out48"] for core in range(N_CORES)])
    return out, res


def kernel(feat, w1, b1, w2, b2, out_h, out_w):
    out, _ = run(feat, w1, b1, w2, b2, out_h, out_w, trace=False)
    return out



# revision 2
# speedup vs baseline: 1.2147x; 1.2147x over previous
"""MetaSR super-resolution Trainium2 kernel (bf16 v2).

Structure exploited: out_h=out_w=256 with H=W=64 LR grid means the scale
factor is exactly 4, so the nearest-neighbor gather index is iy=oy//4,
ix=ox//4 and the per-query MLP input collapses to 16 distinct subpixel
phases [dy/4, dx/4, 0.25].  The whole model becomes:

  1. h    = relu(mlp_in @ w1 + b1)              [16, 256]
  2. predw = h @ w2 + b2                        [16, 576, 3]
  3. rgb[o, 4*iy+dy, 4*ix+dx] =
       sum_{c,ki,kj} feat[c, iy+ki-1, ix+kj-1] * predw[(dy,dx), c*9+ki*3+kj, o]
     i.e. a 3x3 conv with 64 in / 48 out channels + pixel shuffle.

Sharding: data-parallel over LR rows (8 rows per core, 10-row halo band),
weights replicated; steps 1+2 are recomputed on every core (tiny).

The conv contraction (K = 9 taps x 64 ch = 576) is chunked K=128 by pairing
taps.  Each core holds the zero-padded band twice in a 128-partition tile at
free-dim offsets that differ by the two taps' shift delta, so one K=128
matmul consumes two taps without materializing the unfolded tensor:
  band free index = r*66 + x  (66-wide zero-padded rows), tap (ki,kj) shift
  = ki*66+kj; taps are paired with shift deltas 1 or 64.

v2 changes vs the fp32r baseline (28.3us):
  * weights (w2) and band are bf16: input DMA halves to ~1.25MB/core and
    every W-assembly / conv matmul runs 1-pass on the PE (fp32r = 4-pass).
    Simulated end-to-end rel err 3.1e-3 (gate 2e-2).
  * 5 input DMAs ordered by first use across the two HWDGE rings
    (ACT: w1 -> b1b2+w2m0 -> w2m1+m2 | SP: band -> w2m3+m4).
  * conv matmuls split into two 256-col halves so the PSUM->SBUF copy +
    output DMA of half 0 overlaps the half-1 conv matmuls.
  * rgb output DMA'd as bf16 (host converts to fp32).
"""

import numpy as np
import ml_dtypes

try:
    import concourse.bass as bass
except ImportError:  # fall back to the repo checkout
    import sys
    sys.path.insert(0, "/opt/trn_rl_repo")
    import concourse.bass as bass
import concourse.mybir as mybir
import concourse.tile as tile
from concourse import bacc
from concourse.bass_utils import run_bass_kernel_spmd

F32 = mybir.dt.float32
BF16 = mybir.dt.bfloat16
N_CORES = 8
ROWS_PER_CORE = 8          # LR rows per core
BAND_ROWS = ROWS_PER_CORE + 2
NPOS = ROWS_PER_CORE * 64  # 512 LR positions per core

# Tap order for K-chunking.  Taps t = ki*3+kj have band shift ki*66+kj:
#   t:      0   1   2   3    4    5    6    7    8
#   shift:  0   1   2   66   67   68   132  133  134
# chunk0: [t0; t1] band1 off 1 | chunk1: [t3; t2] band2 off 66
# chunk2: [t4; t5] band1 off 68 | chunk3: [t6; t7] band1 off 133
# chunk4: [t8] band2 off 134 (K=64)
TAP_ORDER = [0, 1, 3, 2, 4, 5, 6, 7, 8]
CHUNK_SPECS = [  # (band_tile_idx, rhs_offset, K)
    (0, 1, 128),
    (1, 66, 128),
    (0, 68, 128),
    (0, 133, 128),
    (1, 134, 64),
]

# blob_a0 [3, 544] bf16: w1 [3,256] f32 (cols 0:512) + mlpin [3,16] f32
COLS_A0 = 544
# blob_a1 [128, 34+768]: b1b2 [128,17] f32 (bf16 cols 0:34) + w2 m=0 block
OFF_M0 = 34
COLS_A1 = 34 + 768
# blob_a2 [128, 1536]: w2 m=1,2 blocks
COLS_A2 = 1536
# blob_b0: band1 [128, 661] + band2 [128, 724]
OFF_BAND1 = 0
OFF_BAND2 = 661
COLS_B0 = 1385
# blob_b1 [128, 1152]: w2 m=3 (768) + m=4 (384)
COLS_B1 = 1152

N_WARMUP_MM = 4

_CACHE = {}


def _build_program():
    """Build + compile the single-core Bass program (same for all cores)."""
    nc = bacc.Bacc("TRN2", target_bir_lowering=False, debug=False)

    blob_a0_d = nc.dram_tensor("blob_a0", [3, COLS_A0], BF16, kind="ExternalInput")
    blob_a1_d = nc.dram_tensor("blob_a1", [128, COLS_A1], BF16, kind="ExternalInput")
    blob_a2_d = nc.dram_tensor("blob_a2", [128, COLS_A2], BF16, kind="ExternalInput")
    blob_b0_d = nc.dram_tensor("blob_b0", [128, COLS_B0], BF16, kind="ExternalInput")
    blob_b1_d = nc.dram_tensor("blob_b1", [128, COLS_B1], BF16, kind="ExternalInput")
    out48 = nc.dram_tensor("out48", [48, NPOS], BF16, kind="ExternalOutput")

    with tile.TileContext(nc) as tc:
        with (
            tc.tile_pool(name="blobs", bufs=1) as blobs,
            tc.tile_pool(name="work", bufs=1) as work,
            tc.tile_pool(name="wpool", bufs=5) as wpool,
            tc.tile_pool(name="ps_small", bufs=2, space="PSUM") as ps_small,
            tc.tile_pool(name="ps_w", bufs=5, space="PSUM") as ps_w,
            tc.tile_pool(name="ps_rgb", bufs=1, space="PSUM") as ps_rgb,
        ):
            # DMAs in first-use order: ACT ring (scalar) carries the MLP
            # consts + early w2 blocks, SP ring (sync) carries band + late w2.
            blob_a0 = blobs.tile([3, COLS_A0], BF16, tag="blob_a0")
            nc.scalar.dma_start(blob_a0[:, :], blob_a0_d[:, :])
            blob_b0 = blobs.tile([128, COLS_B0], BF16, tag="blob_b0")
            nc.sync.dma_start(blob_b0[:, :], blob_b0_d[:, :])
            blob_a1 = blobs.tile([128, COLS_A1], BF16, tag="blob_a1")
            nc.scalar.dma_start(blob_a1[:, :], blob_a1_d[:, :])
            blob_b1 = blobs.tile([128, COLS_B1], BF16, tag="blob_b1")
            nc.sync.dma_start(blob_b1[:, :], blob_b1_d[:, :])
            blob_a2 = blobs.tile([128, COLS_A2], BF16, tag="blob_a2")
            nc.scalar.dma_start(blob_a2[:, :], blob_a2_d[:, :])

            a0_f32 = blob_a0.bitcast(F32)           # [3, 272]
            w1_sb = a0_f32[0:3, 0:256]
            mlp_sb = a0_f32[0:3, 256:272]
            b1b2 = blob_a1.bitcast(F32)[:, 0:17]    # [128, 17]
            band_tiles = [
                blob_b0[:, OFF_BAND1:OFF_BAND1 + 661],
                blob_b0[:, OFF_BAND2:OFF_BAND2 + 724],
            ]

            def w2_slice(m, o, hc, msize):
                if m == 0:
                    base = OFF_M0 + (o * 2 + hc) * 128
                    return blob_a1[:, base:base + msize]
                if m <= 2:
                    base = (m - 1) * 768 + (o * 2 + hc) * msize
                    return blob_a2[:, base:base + msize]
                base = (m - 3) * 768 + (o * 2 + hc) * msize
                return blob_b1[:, base:base + msize]

            # ---- PE warm-up: dummy zero matmuls into rgb_ps while DMAs run.
            # conv chunk 0 below uses start=True, which resets the PSUM
            # accumulation, so these contribute nothing to the result.
            rgb_ps = ps_rgb.tile([48, NPOS], F32, tag="rgb")
            warm = work.tile([128, 512], BF16, tag="warm")
            nc.vector.memset(warm[:, :], 0.0)
            for _ in range(N_WARMUP_MM):
                nc.tensor.matmul(
                    rgb_ps[:, :], warm[:, 0:48], warm[:, 0:NPOS],
                    start=True, stop=True,
                )

            # ---- MLP layer 1: h_actT [256, 16] in two 128-chunks ----
            h_sb = work.tile([128, 32], BF16, tag="hact")
            for hc in range(2):
                ph = ps_small.tile([128, 16], F32, tag="ph")
                nc.tensor.matmul(
                    ph[:, :], w1_sb[:, hc * 128:(hc + 1) * 128], mlp_sb[:, :],
                    start=True, stop=True,
                )
                # relu(x + b1) = max(x + b1, 0) in one DVE op
                nc.vector.tensor_scalar(
                    h_sb[:, hc * 16:(hc + 1) * 16], ph[:, :],
                    b1b2[:, hc:hc + 1], 0.0,
                    mybir.AluOpType.add, mybir.AluOpType.max,
                )

            # ---- per K-chunk: W assembly (MLP layer 2) + conv matmuls ----
            w_tiles = []
            for m, (bidx, roff, K) in enumerate(CHUNK_SPECS):
                msize = K
                w_sb = wpool.tile([128, 48], BF16, tag="W")
                for o in range(3):
                    pw = ps_w.tile([128, 16], F32, tag="pw")
                    for hc in range(2):
                        nc.tensor.matmul(
                            pw[:msize, :],
                            w2_slice(m, o, hc, msize),
                            h_sb[:, hc * 16:(hc + 1) * 16],
                            start=(hc == 0), stop=(hc == 1),
                        )
                    nc.vector.tensor_scalar_add(
                        w_sb[:msize, o * 16:(o + 1) * 16], pw[:msize, :],
                        b1b2[:msize, 2 + o * 5 + m:3 + o * 5 + m],
                    )
                w_tiles.append(w_sb)
                # conv half 0 (LR band rows 0-3) right after this chunk's W
                bt = band_tiles[bidx]
                rhs = bt[0:K, roff:roff + 264].rearrange(
                    "p (r c) -> p r c", c=66
                )[:, :, 0:64]
                nc.tensor.matmul(
                    rgb_ps[:, 0:256], w_sb[:msize, :], rhs,
                    start=(m == 0), stop=(m == len(CHUNK_SPECS) - 1),
                )

            # ---- conv half 1 (LR band rows 4-7) ----
            for m, (bidx, roff, K) in enumerate(CHUNK_SPECS):
                bt = band_tiles[bidx]
                rhs = bt[0:K, roff + 264:roff + 528].rearrange(
                    "p (r c) -> p r c", c=66
                )[:, :, 0:64]
                nc.tensor.matmul(
                    rgb_ps[:, 256:512], w_tiles[m][:K, :], rhs,
                    start=(m == 0), stop=(m == len(CHUNK_SPECS) - 1),
                )

            # ---- write out: half 0 overlaps half-1 convs ----
            out_sb = work.tile([48, NPOS], BF16, tag="out")
            nc.vector.tensor_copy(out_sb[:, 0:256], rgb_ps[:, 0:256])
            nc.scalar.dma_start(out48[:, 0:256], out_sb[:, 0:256])
            nc.vector.tensor_copy(out_sb[:, 256:512], rgb_ps[:, 256:512])
            nc.sync.dma_start(out48[:, 256:512], out_sb[:, 256:512])

    nc.compile()
    return nc


def _bf16(x):
    return np.asarray(x, dtype=np.float32).astype(ml_dtypes.bfloat16)


def _host_prep(feat, w1, b1, w2, b2):
    """Pack shared blobs + per-core band blobs (bf16)."""
    feat = np.ascontiguousarray(np.asarray(feat, dtype=np.float32))[0]  # [64,64,64]
    w1 = np.asarray(w1, dtype=np.float32)
    b1 = np.asarray(b1, dtype=np.float32)
    w2 = np.asarray(w2, dtype=np.float32)
    b2 = np.asarray(b2, dtype=np.float32)

    dydx = np.arange(16)
    mlpin = np.stack(
        [dydx // 4 / 4.0, dydx % 4 / 4.0, np.full(16, 0.25)], axis=0
    ).astype(np.float32)  # [3, 16]

    # tap-major permutations of w2/b2
    w2r = w2.reshape(256, 64, 9, 3)  # [h, c, t, o]
    w2p = np.empty((3, 256, 576), dtype=np.float32)
    b2r = b2.reshape(64, 9, 3)       # [c, t, o]
    b2p = np.empty((3, 576), dtype=np.float32)
    for blk, t in enumerate(TAP_ORDER):
        w2p[:, :, blk * 64:(blk + 1) * 64] = w2r[:, :, t, :].transpose(2, 0, 1)
        b2p[:, blk * 64:(blk + 1) * 64] = b2r[:, t, :].T
    w2p = _bf16(w2p)

    b1b2 = np.zeros((128, 17), dtype=np.float32)
    b1b2[:, 0] = b1[0:128]
    b1b2[:, 1] = b1[128:256]
    for o in range(3):
        for m in range(5):
            msize = 128 if m < 4 else 64
            b1b2[:msize, 2 + o * 5 + m] = b2p[o, 128 * m:128 * m + msize]

    blob_a0 = np.zeros((3, COLS_A0), dtype=ml_dtypes.bfloat16)
    a0_u16 = blob_a0.view(np.uint16)
    a0_u16[:, 0:512] = w1.view(np.uint16)
    a0_u16[:, 512:544] = mlpin.view(np.uint16)

    blob_a1 = np.zeros((128, COLS_A1), dtype=ml_dtypes.bfloat16)
    blob_a1.view(np.uint16)[:, 0:34] = b1b2.view(np.uint16)
    for o in range(3):
        for hc in range(2):
            base = OFF_M0 + (o * 2 + hc) * 128
            blob_a1[:, base:base + 128] = w2p[o, hc * 128:(hc + 1) * 128, 0:128]

    blob_a2 = np.empty((128, COLS_A2), dtype=ml_dtypes.bfloat16)
    blob_b1 = np.empty((128, COLS_B1), dtype=ml_dtypes.bfloat16)
    for m in range(1, 5):
        msize = 128 if m < 4 else 64
        dst, moff = (blob_a2, (m - 1) * 768) if m <= 2 else (blob_b1, (m - 3) * 768)
        for o in range(3):
            for hc in range(2):
                base = moff + (o * 2 + hc) * msize
                dst[:, base:base + msize] = \
                    w2p[o, hc * 128:(hc + 1) * 128, 128 * m:128 * m + msize]

    featp = np.zeros((64, 66, 66), dtype=np.float32)
    featp[:, 1:65, 1:65] = feat
    featp = _bf16(featp)

    blobs_b0 = []
    for core in range(N_CORES):
        r0 = core * ROWS_PER_CORE
        band = featp[:, r0:r0 + BAND_ROWS, :].reshape(64, BAND_ROWS * 66)
        bb = np.zeros((128, COLS_B0), dtype=ml_dtypes.bfloat16)
        bb[0:64, OFF_BAND1 + 1:OFF_BAND1 + 661] = band
        bb[64:128, OFF_BAND1 + 0:OFF_BAND1 + 660] = band
        bb[0:64, OFF_BAND2 + 0:OFF_BAND2 + 660] = band
        bb[64:128, OFF_BAND2 + 64:OFF_BAND2 + 724] = band
        blobs_b0.append(bb)
    return blob_a0, blob_a1, blob_a2, blobs_b0, blob_b1


def _assemble(per_core_out48):
    """[8 x [48, 512]] -> [1, 3, 256, 256]."""
    full = np.stack([np.asarray(x, dtype=np.float32) for x in per_core_out48])
    full = full.reshape(8, 3, 4, 4, 8, 64)               # [core, o, dy, dx, r, x]
    rgb = full.transpose(1, 0, 4, 2, 5, 3).reshape(3, 256, 256)
    return np.ascontiguousarray(rgb)[None]


def get_program():
    if "nc" not in _CACHE:
        _CACHE["nc"] = _build_program()
    return _CACHE["nc"]


def run(feat, w1, b1, w2, b2, out_h, out_w, trace=False, **spmd_kwargs):
    assert int(out_h) == 256 and int(out_w) == 256
    nc = get_program()
    blob_a0, blob_a1, blob_a2, blobs_b0, blob_b1 = _host_prep(
        feat, w1, b1, w2, b2
    )
    in_maps = [
        {"blob_a0": blob_a0, "blob_a1": blob_a1, "blob_a2": blob_a2,
         "blob_b0": blobs_b0[core], "blob_b1": blob_b1}
        for core in range(N_CORES)
    ]
    res = run_bass_kernel_spmd(
        nc, in_maps, core_ids=list(range(N_CORES)), trace=trace, **spmd_kwargs
    )
    out = _assemble([res.results[core]["out48"] for core in range(N_CORES)])
    return out, res


def kernel(feat, w1, b1, w2, b2, out_h, out_w):
    out, _ = run(feat, w1, b1, w2, b2, out_h, out_w, trace=False)
    return out


# revision 3
# speedup vs baseline: 1.2964x; 1.0673x over previous
"""MetaSR super-resolution Trainium2 kernel (bf16 v3).

Structure exploited: out_h=out_w=256 with H=W=64 LR grid means the scale
factor is exactly 4, so the nearest-neighbor gather index is iy=oy//4,
ix=ox//4 and the per-query MLP input collapses to 16 distinct subpixel
phases [dy/4, dx/4, 0.25].  The whole model becomes:

  1. h    = relu(mlp_in @ w1 + b1)              [16, 256]
  2. predw = h @ w2 + b2                        [16, 576, 3]
  3. rgb[o, 4*iy+dy, 4*ix+dx] =
       sum_{c,ki,kj} feat[c, iy+ki-1, ix+kj-1] * predw[(dy,dx), c*9+ki*3+kj, o]
     i.e. a 3x3 conv with 64 in / 48 out channels + pixel shuffle.

Sharding: data-parallel over LR rows (8 rows per core, 10-row halo band),
weights replicated; steps 1+2 are recomputed on every core (tiny).

The conv contraction (K = 9 taps x 64 ch = 576) is chunked K=128 by pairing
taps; each core holds the zero-padded band twice in a 128-partition tile at
free-dim offsets differing by the tap-pair shift delta (see CHUNK_SPECS).

All tensors bf16 (simulated end-to-end rel err 4.0e-3, gate 2e-2):
every matmul runs 1-pass on the PE and input DMA is ~1.25MB/core.
DMA order is tuned so each blob lands just before its first PE use:
  SP ring:  small (b1b2+w1+mlpin) -> band1 -> band2 -> w2 m3+m4
  ACT ring: w2 m0 -> m1 -> m2
The two output halves are cast PSUM->SBUF on vector and scalar in
parallel, then DMA'd as one bf16 transfer (host converts to fp32).
"""

import numpy as np
import ml_dtypes

try:
    import concourse.bass as bass
except ImportError:  # fall back to the repo checkout
    import sys
    sys.path.insert(0, "/opt/trn_rl_repo")
    import concourse.bass as bass
import concourse.mybir as mybir
import concourse.tile as tile
from concourse import bacc
from concourse.bass_utils import run_bass_kernel_spmd

F32 = mybir.dt.float32
BF16 = mybir.dt.bfloat16
N_CORES = 8
ROWS_PER_CORE = 8          # LR rows per core
BAND_ROWS = ROWS_PER_CORE + 2
NPOS = ROWS_PER_CORE * 64  # 512 LR positions per core

# Tap order for K-chunking.  Taps t = ki*3+kj have band shift ki*66+kj:
#   t:      0   1   2   3    4    5    6    7    8
#   shift:  0   1   2   66   67   68   132  133  134
# chunk0: [t0; t1] band1 off 1 | chunk1: [t3; t2] band2 off 66
# chunk2: [t4; t5] band1 off 68 | chunk3: [t6; t7] band1 off 133
# chunk4: [t8] band2 off 134 (K=64)
TAP_ORDER = [0, 1, 3, 2, 4, 5, 6, 7, 8]
CHUNK_SPECS = [  # (band_tile_idx, rhs_offset, K)
    (0, 1, 128),
    (1, 66, 128),
    (0, 68, 128),
    (0, 133, 128),
    (1, 134, 64),
]

# small [128, 306] bf16: b1b2 [128,17] f32 (bf16 cols 0:34),
# w1 [3,256] bf16 (rows 0-2, cols 34:290), mlpin [3,16] bf16 (cols 290:306)
OFF_W1 = 34
OFF_MLP = 290
COLS_SMALL = 306
COLS_BAND1 = 661
COLS_BAND2 = 724
COLS_M = 768      # one w2 m-block: 6 sub-blocks (o*2+hc) x [128, 128]
COLS_M34 = 1152   # m3 (768) + m4 (384)

N_WARMUP_MM = 2

_CACHE = {}


def _build_program():
    """Build + compile the single-core Bass program (same for all cores)."""
    nc = bacc.Bacc("TRN2", target_bir_lowering=False, debug=False)

    small_d = nc.dram_tensor("small", [128, COLS_SMALL], BF16, kind="ExternalInput")
    band1_d = nc.dram_tensor("band1", [128, COLS_BAND1], BF16, kind="ExternalInput")
    band2_d = nc.dram_tensor("band2", [128, COLS_BAND2], BF16, kind="ExternalInput")
    m0_d = nc.dram_tensor("m0", [128, COLS_M], BF16, kind="ExternalInput")
    m1_d = nc.dram_tensor("m1", [128, COLS_M], BF16, kind="ExternalInput")
    m2_d = nc.dram_tensor("m2", [128, COLS_M], BF16, kind="ExternalInput")
    m34_d = nc.dram_tensor("m34", [128, COLS_M34], BF16, kind="ExternalInput")
    out48 = nc.dram_tensor("out48", [48, NPOS], BF16, kind="ExternalOutput")

    with tile.TileContext(nc) as tc:
        with (
            tc.tile_pool(name="blobs", bufs=1) as blobs,
            tc.tile_pool(name="work", bufs=1) as work,
            tc.tile_pool(name="wpool", bufs=5) as wpool,
            tc.tile_pool(name="ps_small", bufs=2, space="PSUM") as ps_small,
            tc.tile_pool(name="ps_w", bufs=5, space="PSUM") as ps_w,
            tc.tile_pool(name="ps_rgb", bufs=1, space="PSUM") as ps_rgb,
        ):
            # DMAs in first-use order on each HWDGE ring.
            small = blobs.tile([128, COLS_SMALL], BF16, tag="small")
            nc.sync.dma_start(small[:, :], small_d[:, :])
            m0 = blobs.tile([128, COLS_M], BF16, tag="m0")
            nc.scalar.dma_start(m0[:, :], m0_d[:, :])
            band1 = blobs.tile([128, COLS_BAND1], BF16, tag="band1")
            nc.sync.dma_start(band1[:, :], band1_d[:, :])
            m1 = blobs.tile([128, COLS_M], BF16, tag="m1")
            nc.scalar.dma_start(m1[:, :], m1_d[:, :])
            band2 = blobs.tile([128, COLS_BAND2], BF16, tag="band2")
            nc.sync.dma_start(band2[:, :], band2_d[:, :])
            m2 = blobs.tile([128, COLS_M], BF16, tag="m2")
            nc.scalar.dma_start(m2[:, :], m2_d[:, :])
            m34 = blobs.tile([128, COLS_M34], BF16, tag="m34")
            nc.sync.dma_start(m34[:, :], m34_d[:, :])

            b1b2 = small.bitcast(F32)[:, 0:17]
            w1_sb = small[0:3, OFF_W1:OFF_W1 + 256]
            mlp_sb = small[0:3, OFF_MLP:OFF_MLP + 16]
            band_tiles = [band1, band2]
            mblobs = [m0, m1, m2]

            def w2_slice(m, o, hc, msize):
                if m <= 2:
                    return mblobs[m][:, (o * 2 + hc) * 128:(o * 2 + hc) * 128 + msize]
                base = (m - 3) * 768 + (o * 2 + hc) * msize
                return m34[:, base:base + msize]

            # ---- PE warm-up: dummy zero matmuls into rgb_ps while DMAs run.
            # conv chunk 0 below uses start=True, which resets the PSUM
            # accumulation, so these contribute nothing to the result.
            rgb_ps = ps_rgb.tile([48, NPOS], F32, tag="rgb")
            warm = work.tile([128, 512], BF16, tag="warm")
            nc.vector.memset(warm[:, :], 0.0)
            for _ in range(N_WARMUP_MM):
                nc.tensor.matmul(
                    rgb_ps[:, :], warm[:, 0:48], warm[:, 0:NPOS],
                    start=True, stop=True,
                )

            # ---- MLP layer 1: h_actT [256, 16] in two 128-chunks ----
            h_sb = work.tile([128, 32], BF16, tag="hact")
            for hc in range(2):
                ph = ps_small.tile([128, 16], F32, tag="ph")
                nc.tensor.matmul(
                    ph[:, :], w1_sb[:, hc * 128:(hc + 1) * 128], mlp_sb[:, :],
                    start=True, stop=True,
                )
                # relu(x + b1) = max(x + b1, 0) in one DVE op
                nc.vector.tensor_scalar(
                    h_sb[:, hc * 16:(hc + 1) * 16], ph[:, :],
                    b1b2[:, hc:hc + 1], 0.0,
                    mybir.AluOpType.add, mybir.AluOpType.max,
                )

            # ---- per K-chunk: W assembly (MLP layer 2) + conv matmul ----
            for m, (bidx, roff, K) in enumerate(CHUNK_SPECS):
                msize = K
                w_sb = wpool.tile([128, 48], BF16, tag="W")
                for o in range(3):
                    pw = ps_w.tile([128, 16], F32, tag="pw")
                    for hc in range(2):
                        nc.tensor.matmul(
                            pw[:msize, :],
                            w2_slice(m, o, hc, msize),
                            h_sb[:, hc * 16:(hc + 1) * 16],
                            start=(hc == 0), stop=(hc == 1),
                        )
                    nc.vector.tensor_scalar_add(
                        w_sb[:msize, o * 16:(o + 1) * 16], pw[:msize, :],
                        b1b2[:msize, 2 + o * 5 + m:3 + o * 5 + m],
                    )
                bt = band_tiles[bidx]
                rhs = bt[0:K, roff:roff + 528].rearrange(
                    "p (r c) -> p r c", c=66
                )[:, :, 0:64]
                nc.tensor.matmul(
                    rgb_ps[:, :], w_sb[:msize, :], rhs,
                    start=(m == 0), stop=(m == len(CHUNK_SPECS) - 1),
                )

            # ---- write out: halves cast on vector+scalar in parallel ----
            out_sb = work.tile([48, NPOS], BF16, tag="out")
            nc.vector.tensor_copy(out_sb[:, 0:256], rgb_ps[:, 0:256])
            nc.scalar.activation(
                out_sb[:, 256:512], rgb_ps[:, 256:512],
                mybir.ActivationFunctionType.Copy,
            )
            nc.sync.dma_start(out48[:, :], out_sb[:, :])

    nc.compile()
    return nc


def _bf16(x):
    return np.asarray(x, dtype=np.float32).astype(ml_dtypes.bfloat16)


def _host_prep(feat, w1, b1, w2, b2):
    """Pack shared blobs + per-core band blobs (bf16)."""
    feat = np.ascontiguousarray(np.asarray(feat, dtype=np.float32))[0]  # [64,64,64]
    w1 = np.asarray(w1, dtype=np.float32)
    b1 = np.asarray(b1, dtype=np.float32)
    w2 = np.asarray(w2, dtype=np.float32)
    b2 = np.asarray(b2, dtype=np.float32)

    dydx = np.arange(16)
    mlpin = np.stack(
        [dydx // 4 / 4.0, dydx % 4 / 4.0, np.full(16, 0.25)], axis=0
    ).astype(np.float32)  # [3, 16]

    # tap-major permutations of w2/b2
    w2r = w2.reshape(256, 64, 9, 3)  # [h, c, t, o]
    w2p = np.empty((3, 256, 576), dtype=np.float32)
    b2r = b2.reshape(64, 9, 3)       # [c, t, o]
    b2p = np.empty((3, 576), dtype=np.float32)
    for blk, t in enumerate(TAP_ORDER):
        w2p[:, :, blk * 64:(blk + 1) * 64] = w2r[:, :, t, :].transpose(2, 0, 1)
        b2p[:, blk * 64:(blk + 1) * 64] = b2r[:, t, :].T
    w2p = _bf16(w2p)

    b1b2 = np.zeros((128, 17), dtype=np.float32)
    b1b2[:, 0] = b1[0:128]
    b1b2[:, 1] = b1[128:256]
    for o in range(3):
        for m in range(5):
            msize = 128 if m < 4 else 64
            b1b2[:msize, 2 + o * 5 + m] = b2p[o, 128 * m:128 * m + msize]

    small = np.zeros((128, COLS_SMALL), dtype=ml_dtypes.bfloat16)
    small.view(np.uint16)[:, 0:34] = b1b2.view(np.uint16)
    small[0:3, OFF_W1:OFF_W1 + 256] = _bf16(w1)
    small[0:3, OFF_MLP:OFF_MLP + 16] = _bf16(mlpin)

    mblobs = [np.empty((128, COLS_M), dtype=ml_dtypes.bfloat16) for _ in range(3)]
    m34 = np.empty((128, COLS_M34), dtype=ml_dtypes.bfloat16)
    for m in range(5):
        msize = 128 if m < 4 else 64
        dst, moff = (mblobs[m], 0) if m <= 2 else (m34, (m - 3) * 768)
        for o in range(3):
            for hc in range(2):
                base = moff + (o * 2 + hc) * msize
                dst[:, base:base + msize] = \
                    w2p[o, hc * 128:(hc + 1) * 128, 128 * m:128 * m + msize]

    featp = np.zeros((64, 66, 66), dtype=np.float32)
    featp[:, 1:65, 1:65] = feat
    featp = _bf16(featp)

    bands1, bands2 = [], []
    for core in range(N_CORES):
        r0 = core * ROWS_PER_CORE
        band = featp[:, r0:r0 + BAND_ROWS, :].reshape(64, BAND_ROWS * 66)
        bb1 = np.zeros((128, COLS_BAND1), dtype=ml_dtypes.bfloat16)
        bb1[0:64, 1:661] = band
        bb1[64:128, 0:660] = band
        bb2 = np.zeros((128, COLS_BAND2), dtype=ml_dtypes.bfloat16)
        bb2[0:64, 0:660] = band
        bb2[64:128, 64:724] = band
        bands1.append(bb1)
        bands2.append(bb2)
    return small, mblobs, m34, bands1, bands2


def _assemble(per_core_out48):
    """[8 x [48, 512]] -> [1, 3, 256, 256]."""
    full = np.stack([np.asarray(x, dtype=np.float32) for x in per_core_out48])
    full = full.reshape(8, 3, 4, 4, 8, 64)               # [core, o, dy, dx, r, x]
    rgb = full.transpose(1, 0, 4, 2, 5, 3).reshape(3, 256, 256)
    return np.ascontiguousarray(rgb)[None]


def get_program():
    if "nc" not in _CACHE:
        _CACHE["nc"] = _build_program()
    return _CACHE["nc"]


def run(feat, w1, b1, w2, b2, out_h, out_w, trace=False, **spmd_kwargs):
    assert int(out_h) == 256 and int(out_w) == 256
    nc = get_program()
    small, mblobs, m34, bands1, bands2 = _host_prep(feat, w1, b1, w2, b2)
    in_maps = [
        {"small": small, "m0": mblobs[0], "m1": mblobs[1], "m2": mblobs[2],
         "m34": m34, "band1": bands1[core], "band2": bands2[core]}
        for core in range(N_CORES)
    ]
    res = run_bass_kernel_spmd(
        nc, in_maps, core_ids=list(range(N_CORES)), trace=trace, **spmd_kwargs
    )
    out = _assemble([res.results[core]["out48"] for core in range(N_CORES)])
    return out, res


def kernel(feat, w1, b1, w2, b2, out_h, out_w):
    out, _ = run(feat, w1, b1, w2, b2, out_h, out_w, trace=False)
    return out
